# revision 43
# baseline (speedup 1.0000x reference)
"""GQA causal attention with rope, 8-way head tensor-parallel on one TRN2 chip.

Sharding (per core c of 8): q-heads 4c..4c+3 and kv-head c (kv-head groups kept
intact per the 8 kv heads). Each core computes its heads' attention plus the
partial output projection through its 256-column block of wo; partials are
summed on the host.

Host prep (free): x pre-transposed/pre-tiled to x^T tiles and cast to bf16;
wq/wk rows permuted to [even, odd] rope pairs so rope runs on 32-column blocks;
w_qkv concatenated per core; wo column-block transposed; freqs re-tiled.

Device pipeline per core (Tile framework, bf16 matmuls, fp32 accumulation):
  A) QKV projection on TensorE (x^T tiles stationary, fused q|k|v weights),
     rope on VectorE/GpSimd via block APs with free-dim-broadcast cos/sin,
     PE transposes of q/k into q^T/k^T layout (k duplicated to both partition
     halves). Emission is split into mm / tp / cp filler units so the in-order
     PE queue never waits on the rope chain; strip matmuls and o-proj dots are
     queued between them.
  B) Attention per (pair of heads, 512-wide qs chunk), ascending chunk size:
     k-tiles processed in CLUSTERS OF TWO (both PV pairs, then both scores
     pairs, then both exps) to halve the 64-deep<->128-deep stationary-swap
     transitions; scores S^T = K Q^T as row-tiled 64x128 matmul pairs (both
     heads concurrent in the PE array) into a 2-bank PSUM tile; paired exp on
     ScalarE trimmed to the unmasked range (masked tri via gpsimd
     affine_select); P^T V accumulates into per-head single-bank u tiles so
     the next strip's PV h0 only waits norm(h0), not the whole norm; the
     denominator reciprocal via the DVE 32x32 stream-transpose dance (16
     elems/lane); at the strip tail h1's PVs run on the PE while h0's norm
     runs on the DVE.
  C) Output projection (per-core wo block) into PSUM (two 1-bank tags
     alternated positionally so consecutive dots double-buffer), evacuated in
     bf16 on alternating Scalar/Vector, DMA'd out as partial out^T; emitted as
     PE filler inside the next strip.
  DMA emission order is tuned for the single hardware queue: w ct-chunk 0,
  first x tile, freqs, remaining w, wo last (re-emitted at the second strip).
  Batch 1's phase A interleaves as filler into batch 0's strips.

Host combine: sum the 8 partial out^T tensors, transpose back to [B, S, D].
"""
import sys
for _p in ("/opt/trn_rl_repo",):
    if _p not in sys.path:
        sys.path.insert(0, _p)

import numpy as np
import ml_dtypes

B, S, DIM = 2, 2048, 2048
NH, NKV, HD = 32, 8, 64
P = 128
ST = S // P          # 16 s-tiles
CT = DIM // P        # 16 contraction tiles
NCORE = 8
HPC = NH // NCORE    # 4 q heads per core
QKV = 384            # 4*64 q + 64 k + 64 v columns
NROPE = 320          # rope'd columns (q + k)
NCH = 4              # qs chunks of 512
CHW = 512

_nc_cache = None


def build_nc():
    import concourse.bass as bass
    import concourse.mybir as mybir
    import concourse.tile as tile
    from concourse import bacc
    from concourse.masks import make_identity

    f32 = mybir.dt.float32
    bf16 = mybir.dt.bfloat16

    nc = bacc.Bacc("TRN2", target_bir_lowering=False)
    xt_d = nc.declare_dram_parameter("xt", [B, ST, P, CT, P], bf16, isOutput=False)
    w_d = nc.declare_dram_parameter("wqkv", [P, CT, QKV], bf16, isOutput=False)
    wo_d = nc.declare_dram_parameter("wo", [P, 2, DIM], bf16, isOutput=False)
    fc_d = nc.declare_dram_parameter("fcos", [P, ST, 32], f32, isOutput=False)
    fs_d = nc.declare_dram_parameter("fsin", [P, ST, 32], f32, isOutput=False)
    out_d = nc.declare_dram_parameter("out", [B, ST, P, NCH, CHW], bf16, isOutput=True)

    AP = bass.AP

    def blocks(t, col0, nblk, bstride=64):
        """AP over `nblk` 32-wide col blocks of 2D tile t starting at col0, stride bstride."""
        a = t if isinstance(t, AP) else t[:]
        return AP(tensor=a.tensor, offset=a.offset + col0, ap=[a.ap[0], [bstride, nblk], [1, 32]])

    def bcast32(a, nblk):
        """Broadcast a [128, 32] AP across nblk col blocks."""
        return AP(tensor=a.tensor, offset=a.offset, ap=[a.ap[0], [0, nblk], [1, 32]])

    with tile.TileContext(nc) as tc:
        with (
            tc.tile_pool(name="const", bufs=1) as cst,
            tc.tile_pool(name="work", bufs=3) as work,
            tc.tile_pool(name="perb", bufs=2) as perb,
            tc.tile_pool(name="pp", bufs=24) as pp,
            tc.tile_pool(name="norm", bufs=3) as norm,
            tc.tile_pool(name="normu", bufs=6) as normu,
            tc.tile_pool(name="outp", bufs=4) as outp,
            tc.tile_pool(name="ps_big", bufs=2, space="PSUM") as ps_big,
            tc.tile_pool(name="ps_sm", bufs=1, space="PSUM") as ps_sm,
            tc.tile_pool(name="ps_u", bufs=1, space="PSUM") as ps_u,
        ):
            # DMA emission order = single-queue service order: put the first
            # QKV matmul's dependencies (w chunk 0, then the first xt tile
            # issued by the first astile) ahead of everything else; wo isn't
            # needed until the first o-proj, so it goes last.
            w_sb = cst.tile([P, CT, QKV], bf16, tag="w")
            nc.sync.dma_start(out=w_sb[:, 0:1, :], in_=w_d[:, 0:1, :])
            fc_sb = cst.tile([P, ST, 32], f32, tag="fc")
            fs_sb = cst.tile([P, ST, 32], f32, tag="fs")
            wo_sb = cst.tile([P, 2, DIM], bf16, tag="wo")

            def emit_late_dmas():
                # trickle w in ct order so the first astile's matmuls pace
                # with the single DMA queue; fc/fs early for the first rope
                nc.sync.dma_start(out=w_sb[:, 1:4, :], in_=w_d[:, 1:4, :])
                nc.sync.dma_start(out=fc_sb[:], in_=fc_d[:])
                nc.sync.dma_start(out=fs_sb[:], in_=fs_d[:])
                for g in range(1, 4):
                    nc.sync.dma_start(out=w_sb[:, 4 * g:4 * (g + 1), :],
                                      in_=w_d[:, 4 * g:4 * (g + 1), :])

            ident = cst.tile([P, P], bf16, tag="id")
            make_identity(nc, ident)


            tiles = {}
            for b in range(B):
                qt01 = perb.tile([P, S], bf16, tag="qt01")
                qt23 = perb.tile([P, S], bf16, tag="qt23")
                ktd = perb.tile([P, S], bf16, tag="ktd")
                v1 = perb.tile([P, ST, P], bf16, tag="v1")
                ao01 = perb.tile([P, S], bf16, tag="ao01")
                ao23 = perb.tile([P, S], bf16, tag="ao23")
                tiles[b] = (qt01, qt23, ktd, v1, ao01, ao23)
                nc.vector.memset(v1[:], 1.0)  # ones col; data cols overwritten below

            # ---- Phase A: QKV projection + rope + transposes (one s-tile) ----
            # Split into an mm-part (QKV matmul + rope chain) and a tp-part
            # (PE transposes of the rope output). The tp-part is emitted one
            # filler slot later so the strip matmuls queued between them keep
            # the PE busy while the rope chain (scalar+DVE+gpsimd) runs;
            # emitting the transposes immediately would stall the in-order PE
            # queue for the rope latency (~2.5us measured).
            pending_qk = {}

            def emit_astile_mm(b, st):
                v1 = tiles[b][3]
                xt = work.tile([P, CT, P], bf16, tag="xt")
                nc.sync.dma_start(out=xt[:], in_=xt_d[b, st])
                pmm = ps_sm.tile([P, CHW], f32, tag="pm")
                for ct in range(CT):
                    nc.tensor.matmul(
                        pmm[:, 0:QKV], lhsT=xt[:, ct, :], rhs=w_sb[:, ct, :],
                        start=(ct == 0), stop=(ct == CT - 1),
                    )
                # stage PSUM->SBUF with one scalar copy so pmm frees fast and
                # rope can split across DVE + GpSimd (which cannot read PSUM)
                pms = work.tile([P, QKV], f32, tag="pms")
                nc.scalar.copy(out=pms[:], in_=pmm[:, 0:QKV])
                pm = pms[:]
                cos_st = fc_sb[:, st, :]
                sin_st = fs_sb[:, st, :]
                tA = work.tile([P, NROPE], f32, tag="tA")
                tB = work.tile([P, NROPE], f32, tag="tB")
                # tA = pm * cos on all 10 rope blocks (q0..q3,k) x (t0,t1)
                nc.vector.tensor_mul(blocks(tA, 0, 10, 32), blocks(pm, 0, 10, 32), bcast32(cos_st, 10))
                # tB[t0 blocks] = pm[t1 blocks] * sin ; tB[t1] = pm[t0] * sin
                nc.gpsimd.tensor_mul(blocks(tB, 0, 5), blocks(pm, 32, 5), bcast32(sin_st, 5))
                nc.gpsimd.tensor_mul(blocks(tB, 32, 5), blocks(pm, 0, 5), bcast32(sin_st, 5))
                qk = work.tile([P, NROPE], bf16, tag="qk")
                nc.vector.tensor_sub(blocks(qk, 0, 5), blocks(tA, 0, 5), blocks(tB, 0, 5))
                nc.vector.tensor_add(blocks(qk, 32, 5), blocks(tA, 32, 5), blocks(tB, 32, 5))
                nc.gpsimd.tensor_copy(v1[:, st, 0:64], pm[:, NROPE:QKV])
                pending_qk[(b, st)] = qk

            pending_tp = {}

            def emit_astile_tp(b, st):
                qk = pending_qk.pop((b, st))
                # PE transposes: q01, q23 [128,128]; k [128,64] - all three into
                # one PSUM bank so they don't rotate through the pm slot
                tp = ps_sm.tile([P, 3, P], bf16, tag="tp")
                nc.tensor.transpose(tp[:, 0, :], qk[:, 0:P], ident[:])
                nc.tensor.transpose(tp[:, 1, :], qk[:, P:2 * P], ident[:])
                nc.tensor.transpose(tp[0:64, 2, :], qk[:, 256:320], ident[:])
                pending_tp[(b, st)] = tp

            def emit_astile_cp(b, st):
                # evac copies emitted one slot after the transposes so the
                # scalar/vector queues never head-of-line block on the PE
                qt01, qt23, ktd = tiles[b][0], tiles[b][1], tiles[b][2]
                tp = pending_tp.pop((b, st))
                cols = slice(st * P, (st + 1) * P)
                nc.scalar.copy(out=qt01[:, cols], in_=tp[:, 0, :])
                nc.vector.tensor_copy(qt23[:, cols], tp[:, 1, :])
                nc.scalar.copy(out=ktd[0:64, cols], in_=tp[0:64, 2, :])
                nc.vector.tensor_copy(ktd[64:128, cols], tp[0:64, 2, :])

            # ---- Phases B+C per batch: strips (desc chunk size), O-proj one chunk behind ----
            def emit_oproj_dot(b, ch, dot, tag="pm", eng=None):
                ao01, ao23 = tiles[b][4], tiles[b][5]
                # tag picked by the scheduler: the bank whose previous user
                # was evacuated longest ago (alternating pm/tp pipelines dots)
                po = ps_sm.tile([P, CHW], f32, tag=tag)
                nc.tensor.matmul(po[:], lhsT=wo_sb[:, 0, dot * P:(dot + 1) * P],
                                 rhs=ao01[:, ch * CHW:(ch + 1) * CHW], start=True, stop=False)
                nc.tensor.matmul(po[:], lhsT=wo_sb[:, 1, dot * P:(dot + 1) * P],
                                 rhs=ao23[:, ch * CHW:(ch + 1) * CHW], start=False, stop=True)
                so = outp.tile([P, CHW], bf16, tag="so")
                if eng == "scalar" or (eng is None and dot % 2 == 0):
                    nc.scalar.copy(out=so[:], in_=po[:])
                else:
                    nc.vector.tensor_copy(so[:], po[:])
                nc.sync.dma_start(out=out_d[b, dot, :, ch, :], in_=so[:])

            # Final-chunk split o-proj: the wo0^T ao01 halves run as single
            # matmuls (bf16-staged to SBUF) as fillers inside the LAST strip,
            # filling the PE right after pair0's norms; the tail then only
            # runs the wo1^T ao23 halves, with the staged half added during
            # the bf16 evacuation (no extra copy).
            stage_po = {}

            def emit_oproj_mm1(b, ch, dot, tag):
                ao01 = tiles[b][4]
                po = ps_sm.tile([P, CHW], f32, name="po1", tag=tag)
                nc.tensor.matmul(po[:], lhsT=wo_sb[:, 0, dot * P:(dot + 1) * P],
                                 rhs=ao01[:, ch * CHW:(ch + 1) * CHW],
                                 start=True, stop=True)
                stg = outp.tile([P, CHW], bf16, name="stg", tag="stg", bufs=16)
                if dot % 2 == 0:
                    nc.scalar.copy(out=stg[:], in_=po[:])
                else:
                    nc.vector.tensor_copy(stg[:], po[:])
                stage_po[(b, ch, dot)] = stg

            def oproj_chunk_final(b, ch):
                ao23 = tiles[b][5]
                for dot in range(0, ST, 2):
                    so2 = outp.tile([P, 2, CHW], bf16, tag="so2")
                    pos = []
                    for j in range(2):
                        pos.append(ps_sm.tile([P, CHW], f32, name=f"pof{j}",
                                   tag=("pm" if j == 0 else "tp")))
                        nc.tensor.matmul(
                            pos[j][:],
                            lhsT=wo_sb[:, 1, (dot + j) * P:(dot + j + 1) * P],
                            rhs=ao23[:, ch * CHW:(ch + 1) * CHW],
                            start=True, stop=True)
                    s0 = stage_po.pop((b, ch, dot))
                    s1 = stage_po.pop((b, ch, dot + 1))
                    nc.scalar.activation(so2[:, 0, :], pos[0][:],
                                         mybir.ActivationFunctionType.Copy,
                                         bias=s0[:])
                    nc.vector.tensor_add(so2[:, 1, :], pos[1][:], s1[:])
                    a = out_d[b, dot:dot + 2, :, ch, :]
                    nc.sync.dma_start(
                        out=AP(tensor=a.tensor, offset=a.offset,
                               ap=[a.ap[1], a.ap[0], a.ap[2]]),
                        in_=so2[:])

            def oproj_chunk(b, ch):
                # Final chunk, 3 dot-pairs in flight: pm/tp banks plus one
                # dead scores slot (2 banks). All wo0-half matmuls of the
                # in-flight window are emitted BEFORE any norm-gated wo1
                # half, so the in-order PE queue fills the last strip's norm
                # latency (~4us) instead of idling behind 2 open banks.
                ao01, ao23 = tiles[b][4], tiles[b][5]
                npair = ST // 2
                banks = {}

                def get_banks(pp):
                    if pp % 3 == 2:
                        big = ps_big.tile([P, 2, CHW], f32, name="pobig",
                                          tag="big")
                        return (big[:, 0, :], big[:, 1, :])
                    a = ps_sm.tile([P, CHW], f32, name="poa",
                                   tag=("pm" if pp % 3 == 0 else "tp"))
                    b2 = ps_sm.tile([P, CHW], f32, name="pob",
                                    tag=("tp" if pp % 3 == 0 else "pm"))
                    return (a[:], b2[:])

                def emit_wo0(pp):
                    banks[pp] = get_banks(pp)
                    for j in range(2):
                        dd = 2 * pp + j
                        nc.tensor.matmul(
                            banks[pp][j],
                            lhsT=wo_sb[:, 0, dd * P:(dd + 1) * P],
                            rhs=ao01[:, ch * CHW:(ch + 1) * CHW],
                            start=True, stop=False)

                def emit_wo1(pp):
                    pos = banks.pop(pp)
                    for j in range(2):
                        dd = 2 * pp + j
                        nc.tensor.matmul(
                            pos[j],
                            lhsT=wo_sb[:, 1, dd * P:(dd + 1) * P],
                            rhs=ao23[:, ch * CHW:(ch + 1) * CHW],
                            start=False, stop=True)
                    so2 = outp.tile([P, 2, CHW], bf16, tag="so2")
                    nc.scalar.copy(out=so2[:, 0, :], in_=pos[0])
                    nc.scalar.copy(out=so2[:, 1, :], in_=pos[1])
                    a = out_d[b, 2 * pp:2 * pp + 2, :, ch, :]
                    nc.sync.dma_start(
                        out=AP(tensor=a.tensor, offset=a.offset,
                               ap=[a.ap[1], a.ap[0], a.ap[2]]),
                        in_=so2[:])

                for pp in range(npair):
                    emit_wo0(pp)
                    if pp >= 2:
                        emit_wo1(pp - 2)
                emit_wo1(npair - 2)
                emit_wo1(npair - 1)

            def strip(b, pair, ch, filler=()):
                qt01, qt23, ktd, v1, ao01, ao23 = tiles[b]
                qt, ao = (qt01, ao01) if pair == 0 else (qt23, ao23)
                nks = 4 * (ch + 1)
                filler = list(filler)
                fsched = [[] for _ in range(nks)]
                for i, fd in enumerate(filler):
                    fsched[(i * nks) // len(filler)].append(fd)
                ppr = []
                # per-head single-bank u tiles: pair N+1's PV h0 only waits
                # for pair N's norm_head(h0), not the whole norm
                u0 = ps_u.tile([P, CHW], f32, tag="u0")
                u1 = ps_u.tile([P, CHW], f32, tag="u1")
                uh = (u0, u1)
                DLY = 4

                def emit_pv(kst):
                    lo = max(0, kst - 4 * ch) * P
                    for h in range(2):
                        nc.tensor.matmul(uh[h][:, lo:CHW], lhsT=v1[:, kst, :],
                                         rhs=ppr[kst][:, h, lo:CHW],
                                         start=(kst == 0), stop=(kst == nks - 1))

                def emit_scores(kst):
                    o = kst - 4 * ch
                    lo = max(0, o) * P  # first live qs column of this k-tile
                    psc = ps_big.tile([P, 2, CHW], f32, tag="big")
                    for h in range(2):
                        nc.tensor.matmul(
                            psc[:, h, lo:CHW],
                            lhsT=ktd[64 * h:64 * h + 64, kst * P:(kst + 1) * P],
                            rhs=qt[64 * h:64 * h + 64, ch * CHW + lo:(ch + 1) * CHW],
                            start=True, stop=True)
                    return psc, lo, o

                def emit_exp(kst, psc, lo, o):
                    pt = pp.tile([P, 2, CHW], bf16, tag="p")
                    nc.scalar.activation(pt[:, :, lo:CHW], psc[:, :, lo:CHW],
                                         mybir.ActivationFunctionType.Exp, scale=0.125)
                    if o >= 0:
                        nc.gpsimd.affine_select(
                            out=pt[:, :, lo:lo + P], in_=pt[:, :, lo:lo + P],
                            compare_op=mybir.AluOpType.is_ge,
                            fill=0.0, base=0, channel_multiplier=-1,
                            pattern=[[0, 2], [1, P]],
                        )
                    ppr.append(pt)

                # Process k-tiles in pairs: both PV pairs, then both scores
                # pairs, then both exps. This halves the number of
                # stationary-swap transitions between the 64-deep scores
                # matmuls and the 128-deep PV matmuls (~200ns each).
                for k0 in range(0, nks, 2):
                    for kk in (k0, k0 + 1):
                        if kk >= DLY:
                            emit_pv(kk - DLY)
                    sa = emit_scores(k0)
                    sb = emit_scores(k0 + 1)
                    emit_exp(k0, *sa)
                    emit_exp(k0 + 1, *sb)
                    for fd in fsched[k0] + fsched[k0 + 1]:
                        fd()

                def norm_head(u, basep):
                    # Denominators arrive replicated on partitions 64:128 (64
                    # ones-columns in v1), so a DVE 32x32 block transpose can
                    # spread them across partitions directly; the reciprocal
                    # then runs 16/lane instead of 512 on one lane.
                    tq = normu.tile([64, CHW], f32, tag="tq")
                    nc.vector.transpose(tq[:], u[64:128, :])
                    rt = norm.tile([64, 16], f32, tag="rt")
                    tqa = tq[:]
                    nc.vector.reciprocal(
                        rt[:], AP(tensor=tqa.tensor, offset=tqa.offset,
                                  ap=[tqa.ap[0], [32, 16]]))
                    rb = normu.tile([64, CHW], f32, tag="rb")
                    rta, rba = rt[:], rb[:]
                    nc.vector.transpose(
                        AP(tensor=rba.tensor, offset=rba.offset,
                           ap=[rba.ap[0], [32, 16], [1, 32]]),
                        AP(tensor=rta.tensor, offset=rta.offset,
                           ap=[rta.ap[0], [1, 16], [0, 32]]))
                    nc.vector.tensor_mul(
                        ao[basep:basep + 64, ch * CHW:(ch + 1) * CHW],
                        u[0:64, :], rb[:])

                # Tail: finish h0's PVs first so h0's norm (DVE) overlaps
                # h1's remaining PV matmuls on the PE.
                def emit_pv_h(kst, h):
                    lo = max(0, kst - 4 * ch) * P
                    nc.tensor.matmul(uh[h][:, lo:CHW], lhsT=v1[:, kst, :],
                                     rhs=ppr[kst][:, h, lo:CHW],
                                     start=(kst == 0), stop=(kst == nks - 1))

                for kst in range(max(0, nks - DLY), nks):
                    emit_pv_h(kst, 0)
                norm_head(u0[:], 0)
                for kst in range(max(0, nks - DLY), nks):
                    emit_pv_h(kst, 1)
                norm_head(u1[:], 64)

            # ---- emission schedule ----
            # Ascending chunks pipeline phase A into the strips: strip (b, ch)
            # only needs A(b) s-tiles 0..4ch+3, so later A s-tiles stream in as
            # PE filler one chunk ahead of their first use.
            astiles = [(b, st) for b in range(B) for st in range(ST)]
            emit_astile_mm(0, 0)
            emit_late_dmas()
            emit_astile_mm(0, 1)
            emit_astile_tp(0, 0)
            emit_astile_mm(0, 2)
            emit_astile_cp(0, 0)
            emit_astile_tp(0, 1)
            emit_astile_mm(0, 3)
            emit_astile_cp(0, 1)
            emit_astile_tp(0, 2)
            emit_astile_cp(0, 2)
            emit_astile_tp(0, 3)
            emit_astile_cp(0, 3)
            ai = 4
            prev = None
            for b in range(B):
                for ch in (0, 1, 2, 3):
                    # A s-tiles needed one chunk ahead (next strip in sequence)
                    if b == 0 and ch < 3:
                        need = 4 + 4 * (ch + 1) + 4
                    elif b == 0:
                        need = ST + 6
                    elif ch < 3:
                        need = ST + min(ST, 4 * (ch + 1) + 4)
                    else:
                        need = 2 * ST
                    take = astiles[ai:need]
                    ai = max(ai, need)
                    for pair in (0, 1):
                        if b == 0 and ch == 0 and pair == 1:
                            # wo is first read by (b0, ch1) oproj fillers;
                            # emitting here keeps it out of the startup queue
                            nc.sync.dma_start(out=wo_sb[:], in_=wo_d[:])
                        dots = []
                        if prev is not None:
                            pb, pch = prev
                            alld = [(pb, pch, d) for d in range(ST)]
                            dots = alld[:8] if pair == 0 else alld[8:]
                            if b == 1 and ch == 3 and pair == 1:
                                # hold back 4 dots: emitted after this strip,
                                # they fill the PE during the final norms
                                deferred_dots = dots[4:]
                                dots = dots[:4]
                        ta = (take[:(len(take) + 1) // 2] if pair == 0
                              else take[(len(take) + 1) // 2:])
                        # Interleave astile mm / oproj dots / tp / cp so the
                        # PE always has dot matmuls queued between an astile's
                        # QKV matmuls and its transposes (which wait on rope).
                        # Dot PSUM tags are positional: a dot emitted right
                        # after mm uses the tp bank (its last user was evac'd
                        # a whole astile ago); dots after cp alternate.
                        filler = []
                        di = 0

                        last = (b == 1 and ch == 3)

                        def dot_f(tag):
                            nonlocal di
                            if di < len(dots):
                                filler.append(
                                    lambda t=dots[di], g=tag,
                                    e=("scalar" if last else None):
                                    emit_oproj_dot(*t, tag=g, eng=e))
                                di += 1
                        if not dots:
                            # No dot matmuls to pad with: emit all the mm
                            # parts first (each covers the previous one's
                            # rope latency), then the tp/cp chains.
                            for (ab, ast) in ta:
                                filler.append(
                                    lambda ab=ab, ast=ast: emit_astile_mm(ab, ast))
                            for (ab, ast) in ta:
                                filler.append(
                                    lambda ab=ab, ast=ast: emit_astile_tp(ab, ast))
                                filler.append(
                                    lambda ab=ab, ast=ast: emit_astile_cp(ab, ast))
                            ta = []
                        for (ab, ast) in ta:
                            filler.append(lambda ab=ab, ast=ast: emit_astile_mm(ab, ast))
                            dot_f("tp")
                            dot_f("pm")
                            filler.append(lambda ab=ab, ast=ast: emit_astile_tp(ab, ast))
                            filler.append(lambda ab=ab, ast=ast: emit_astile_cp(ab, ast))
                            # pm first: the tp bank's evac copies (cp, just
                            # emitted) are still draining through the queues
                            dot_f("pm")
                            dot_f("tp")
                        for j, t in enumerate(dots[di:]):
                            filler.append(lambda t=t, g=("tp" if j % 2 == 0 else "pm"):
                                          emit_oproj_dot(*t, tag=g))
                        strip(b, pair, ch, filler=filler)
                    prev = (b, ch)
            for j, t in enumerate(deferred_dots):
                emit_oproj_dot(*t, tag=("pm" if j % 2 == 0 else "tp"),
                               eng="scalar")
            oproj_chunk(*prev)

    nc.compile()
    return nc


def get_nc():
    global _nc_cache
    if _nc_cache is None:
        _nc_cache = build_nc()
    return _nc_cache


def prep_inputs(x, freqs_cos, freqs_sin, wq, wk, wv, wo):
    """Host-side layout prep. Returns list of per-core input dicts."""
    bf = ml_dtypes.bfloat16
    x = np.asarray(x, dtype=np.float32)
    # xh[b, st, p, ct, sl] = x[b, st*128+sl, ct*128+p]
    xh = np.ascontiguousarray(
        x.reshape(B, ST, P, CT, P).transpose(0, 1, 4, 3, 2).astype(bf))
    # fc[p, st, j] = freqs_cos[st*128+p, j]
    fc = np.ascontiguousarray(
        np.asarray(freqs_cos, np.float32).reshape(ST, P, 32).transpose(1, 0, 2))
    fs = np.ascontiguousarray(
        np.asarray(freqs_sin, np.float32).reshape(ST, P, 32).transpose(1, 0, 2))
    perm = np.concatenate([np.arange(0, HD, 2), np.arange(1, HD, 2)])
    in_maps = []
    for c in range(NCORE):
        q_rows = np.asarray(wq, np.float32)[c * HPC * HD:(c + 1) * HPC * HD]
        q_rows = q_rows.reshape(HPC, HD, DIM)[:, perm, :].reshape(HPC * HD, DIM)
        k_rows = np.asarray(wk, np.float32)[c * HD:(c + 1) * HD][perm]
        v_rows = np.asarray(wv, np.float32)[c * HD:(c + 1) * HD]
        wcat = np.concatenate([q_rows, k_rows, v_rows], axis=0)  # [384, DIM]
        w_h = np.ascontiguousarray(wcat.T.reshape(CT, P, QKV).transpose(1, 0, 2).astype(bf))
        wo_cols = np.asarray(wo, np.float32)[:, c * HPC * HD:(c + 1) * HPC * HD]  # [DIM, 256]
        wo_h = np.ascontiguousarray(wo_cols.T.reshape(2, P, DIM).transpose(1, 0, 2).astype(bf))
        in_maps.append({"xt": xh, "wqkv": w_h, "wo": wo_h, "fcos": fc, "fsin": fs})
    return in_maps


def combine_outputs(results):
    """Sum per-core partial out^T and return [B, S, DIM] float32."""
    acc = np.zeros((B, ST, P, NCH, CHW), np.float32)
    for r in results:
        acc += r["out"].astype(np.float32)
    # out[b, ch*512+sl, dot*128+p] = acc[b, dot, p, ch, sl]
    return np.ascontiguousarray(
        acc.transpose(0, 3, 4, 1, 2).reshape(B, S, DIM).astype(np.float32))


def kernel(x, freqs_cos, freqs_sin, wq, wk, wv, wo):
    from concourse.bass_utils import run_bass_kernel_spmd

    nc = get_nc()
    in_maps = prep_inputs(x, freqs_cos, freqs_sin, wq, wk, wv, wo)
    res = run_bass_kernel_spmd(nc, in_maps, core_ids=list(range(NCORE)))
    return combine_outputs(res.results)



# revision 44
# speedup vs baseline: 1.0152x; 1.0152x over previous
"""GQA causal attention with rope, 8-way head tensor-parallel on one TRN2 chip.

Sharding (per core c of 8): q-heads 4c..4c+3 and kv-head c (kv-head groups kept
intact per the 8 kv heads). Each core computes its heads' attention plus the
partial output projection through its 256-column block of wo; partials are
summed on the host.

Host prep (free): x pre-transposed/pre-tiled to x^T tiles and cast to bf16;
wq/wk rows permuted to [even, odd] rope pairs so rope runs on 32-column blocks;
w_qkv concatenated per core; wo column-block transposed; freqs re-tiled.

Device pipeline per core (Tile framework, bf16 matmuls, fp32 accumulation):
  A) QKV projection on TensorE (x^T tiles stationary, fused q|k|v weights),
     rope on VectorE/GpSimd via block APs with free-dim-broadcast cos/sin,
     PE transposes of q/k into q^T/k^T layout (k duplicated to both partition
     halves). Emission is split into mm / tp / cp filler units so the in-order
     PE queue never waits on the rope chain; strip matmuls and o-proj dots are
     queued between them.
  B) Attention per (pair of heads, 512-wide qs chunk), ascending chunk size:
     k-tiles processed in CLUSTERS OF TWO (both PV pairs, then both scores
     pairs, then both exps) to halve the 64-deep<->128-deep stationary-swap
     transitions; scores S^T = K Q^T as row-tiled 64x128 matmul pairs (both
     heads concurrent in the PE array) into a 2-bank PSUM tile; paired exp on
     ScalarE trimmed to the unmasked range (masked tri via gpsimd
     affine_select); P^T V accumulates into per-head single-bank u tiles so
     the next strip's PV h0 only waits norm(h0), not the whole norm; the
     denominator reciprocal via the DVE 32x32 stream-transpose dance (16
     elems/lane); at the strip tail h1's PVs run on the PE while h0's norm
     runs on the DVE.
  C) Output projection (per-core wo block) into PSUM (two 1-bank tags
     alternated positionally so consecutive dots double-buffer), evacuated in
     bf16 on alternating Scalar/Vector, DMA'd out as partial out^T; emitted as
     PE filler inside the next strip.
  DMA emission order is tuned for the single hardware queue: w ct-chunk 0,
  first x tile, freqs, remaining w, wo last (re-emitted at the second strip).
  Batch 1's phase A interleaves as filler into batch 0's strips.

Host combine: sum the 8 partial out^T tensors, transpose back to [B, S, D].
"""
import sys
for _p in ("/opt/trn_rl_repo",):
    if _p not in sys.path:
        sys.path.insert(0, _p)

import numpy as np
import ml_dtypes

B, S, DIM = 2, 2048, 2048
NH, NKV, HD = 32, 8, 64
P = 128
ST = S // P          # 16 s-tiles
CT = DIM // P        # 16 contraction tiles
NCORE = 8
HPC = NH // NCORE    # 4 q heads per core
QKV = 384            # 4*64 q + 64 k + 64 v columns
NROPE = 320          # rope'd columns (q + k)
NCH = 4              # qs chunks of 512
CHW = 512

_nc_cache = None


def build_nc():
    import concourse.bass as bass
    import concourse.mybir as mybir
    import concourse.tile as tile
    from concourse import bacc
    from concourse.masks import make_identity

    f32 = mybir.dt.float32
    bf16 = mybir.dt.bfloat16

    nc = bacc.Bacc("TRN2", target_bir_lowering=False)
    xt_d = nc.declare_dram_parameter("xt", [B, ST, P, CT, P], bf16, isOutput=False)
    w_d = nc.declare_dram_parameter("wqkv", [P, CT, QKV], bf16, isOutput=False)
    wo_d = nc.declare_dram_parameter("wo", [P, 2, DIM], bf16, isOutput=False)
    fc_d = nc.declare_dram_parameter("fcos", [P, ST, 32], f32, isOutput=False)
    fs_d = nc.declare_dram_parameter("fsin", [P, ST, 32], f32, isOutput=False)
    out_d = nc.declare_dram_parameter("out", [B, ST, P, NCH, CHW], bf16, isOutput=True)

    AP = bass.AP

    def blocks(t, col0, nblk, bstride=64):
        """AP over `nblk` 32-wide col blocks of 2D tile t starting at col0, stride bstride."""
        a = t if isinstance(t, AP) else t[:]
        return AP(tensor=a.tensor, offset=a.offset + col0, ap=[a.ap[0], [bstride, nblk], [1, 32]])

    def bcast32(a, nblk):
        """Broadcast a [128, 32] AP across nblk col blocks."""
        return AP(tensor=a.tensor, offset=a.offset, ap=[a.ap[0], [0, nblk], [1, 32]])

    with tile.TileContext(nc) as tc:
        with (
            tc.tile_pool(name="const", bufs=1) as cst,
            tc.tile_pool(name="work", bufs=3) as work,
            tc.tile_pool(name="perb", bufs=2) as perb,
            tc.tile_pool(name="pp", bufs=24) as pp,
            tc.tile_pool(name="norm", bufs=3) as norm,
            tc.tile_pool(name="normu", bufs=6) as normu,
            tc.tile_pool(name="outp", bufs=4) as outp,
            tc.tile_pool(name="ps_big", bufs=2, space="PSUM") as ps_big,
            tc.tile_pool(name="ps_sm", bufs=1, space="PSUM") as ps_sm,
            tc.tile_pool(name="ps_u", bufs=1, space="PSUM") as ps_u,
        ):
            # DMA emission order = single-queue service order: put the first
            # QKV matmul's dependencies (w chunk 0, then the first xt tile
            # issued by the first astile) ahead of everything else; wo isn't
            # needed until the first o-proj, so it goes last.
            w_sb = cst.tile([P, CT, QKV], bf16, tag="w")
            nc.sync.dma_start(out=w_sb[:, 0:1, :], in_=w_d[:, 0:1, :])
            fc_sb = cst.tile([P, ST, 32], f32, tag="fc")
            fs_sb = cst.tile([P, ST, 32], f32, tag="fs")
            wo_sb = cst.tile([P, 2, DIM], bf16, tag="wo")

            def emit_late_dmas():
                # trickle w in ct order so the first astile's matmuls pace
                # with the single DMA queue; fc/fs early for the first rope
                nc.sync.dma_start(out=w_sb[:, 1:4, :], in_=w_d[:, 1:4, :])
                nc.sync.dma_start(out=fc_sb[:], in_=fc_d[:])
                nc.sync.dma_start(out=fs_sb[:], in_=fs_d[:])
                for g in range(1, 4):
                    nc.sync.dma_start(out=w_sb[:, 4 * g:4 * (g + 1), :],
                                      in_=w_d[:, 4 * g:4 * (g + 1), :])

            ident = cst.tile([P, P], bf16, tag="id")
            make_identity(nc, ident)


            tiles = {}
            for b in range(B):
                qt01 = perb.tile([P, S], bf16, tag="qt01")
                qt23 = perb.tile([P, S], bf16, tag="qt23")
                ktd = perb.tile([P, S], bf16, tag="ktd")
                v1 = perb.tile([P, ST, P], bf16, tag="v1")
                ao01 = perb.tile([P, S], bf16, tag="ao01")
                ao23 = perb.tile([P, S], bf16, tag="ao23")
                tiles[b] = (qt01, qt23, ktd, v1, ao01, ao23)
                nc.vector.memset(v1[:], 1.0)  # ones col; data cols overwritten below

            # ---- Phase A: QKV projection + rope + transposes (one s-tile) ----
            # Split into an mm-part (QKV matmul + rope chain) and a tp-part
            # (PE transposes of the rope output). The tp-part is emitted one
            # filler slot later so the strip matmuls queued between them keep
            # the PE busy while the rope chain (scalar+DVE+gpsimd) runs;
            # emitting the transposes immediately would stall the in-order PE
            # queue for the rope latency (~2.5us measured).
            pending_qk = {}

            def emit_astile_mm(b, st, split_dma=False):
                v1 = tiles[b][3]
                xt = work.tile([P, CT, P], bf16, tag="xt")
                if split_dma:
                    # first tile: chunked so MM(ct0) waits 128KB, not 512KB
                    for g in range(4):
                        nc.sync.dma_start(out=xt[:, 4 * g:4 * (g + 1), :],
                                          in_=xt_d[b, st, :, 4 * g:4 * (g + 1), :])
                else:
                    nc.sync.dma_start(out=xt[:], in_=xt_d[b, st])
                pmm = ps_sm.tile([P, CHW], f32, tag="pm")
                for ct in range(CT):
                    nc.tensor.matmul(
                        pmm[:, 0:QKV], lhsT=xt[:, ct, :], rhs=w_sb[:, ct, :],
                        start=(ct == 0), stop=(ct == CT - 1),
                    )
                # stage PSUM->SBUF with one scalar copy so pmm frees fast and
                # rope can split across DVE + GpSimd (which cannot read PSUM)
                pms = work.tile([P, QKV], f32, tag="pms")
                nc.scalar.copy(out=pms[:], in_=pmm[:, 0:QKV])
                pm = pms[:]
                cos_st = fc_sb[:, st, :]
                sin_st = fs_sb[:, st, :]
                tA = work.tile([P, NROPE], f32, tag="tA")
                tB = work.tile([P, NROPE], f32, tag="tB")
                # tA = pm * cos on all 10 rope blocks (q0..q3,k) x (t0,t1)
                nc.vector.tensor_mul(blocks(tA, 0, 10, 32), blocks(pm, 0, 10, 32), bcast32(cos_st, 10))
                # tB[t0 blocks] = pm[t1 blocks] * sin ; tB[t1] = pm[t0] * sin
                nc.gpsimd.tensor_mul(blocks(tB, 0, 5), blocks(pm, 32, 5), bcast32(sin_st, 5))
                nc.gpsimd.tensor_mul(blocks(tB, 32, 5), blocks(pm, 0, 5), bcast32(sin_st, 5))
                qk = work.tile([P, NROPE], bf16, tag="qk")
                nc.vector.tensor_sub(blocks(qk, 0, 5), blocks(tA, 0, 5), blocks(tB, 0, 5))
                nc.vector.tensor_add(blocks(qk, 32, 5), blocks(tA, 32, 5), blocks(tB, 32, 5))
                nc.gpsimd.tensor_copy(v1[:, st, 0:64], pm[:, NROPE:QKV])
                pending_qk[(b, st)] = qk

            pending_tp = {}

            def emit_astile_tp(b, st):
                qk = pending_qk.pop((b, st))
                # PE transposes: q01, q23 [128,128]; k [128,64] - all three into
                # one PSUM bank so they don't rotate through the pm slot
                tp = ps_sm.tile([P, 3, P], bf16, tag="tp")
                nc.tensor.transpose(tp[:, 0, :], qk[:, 0:P], ident[:])
                nc.tensor.transpose(tp[:, 1, :], qk[:, P:2 * P], ident[:])
                nc.tensor.transpose(tp[0:64, 2, :], qk[:, 256:320], ident[:])
                pending_tp[(b, st)] = tp

            def emit_astile_cp(b, st):
                # evac copies emitted one slot after the transposes so the
                # scalar/vector queues never head-of-line block on the PE
                qt01, qt23, ktd = tiles[b][0], tiles[b][1], tiles[b][2]
                tp = pending_tp.pop((b, st))
                cols = slice(st * P, (st + 1) * P)
                nc.scalar.copy(out=qt01[:, cols], in_=tp[:, 0, :])
                nc.vector.tensor_copy(qt23[:, cols], tp[:, 1, :])
                nc.scalar.copy(out=ktd[0:64, cols], in_=tp[0:64, 2, :])
                nc.vector.tensor_copy(ktd[64:128, cols], tp[0:64, 2, :])

            # ---- Phases B+C per batch: strips (desc chunk size), O-proj one chunk behind ----
            def emit_oproj_dot(b, ch, dot, tag="pm", eng=None):
                ao01, ao23 = tiles[b][4], tiles[b][5]
                # tag picked by the scheduler: the bank whose previous user
                # was evacuated longest ago (alternating pm/tp pipelines dots)
                po = ps_sm.tile([P, CHW], f32, tag=tag)
                nc.tensor.matmul(po[:], lhsT=wo_sb[:, 0, dot * P:(dot + 1) * P],
                                 rhs=ao01[:, ch * CHW:(ch + 1) * CHW], start=True, stop=False)
                nc.tensor.matmul(po[:], lhsT=wo_sb[:, 1, dot * P:(dot + 1) * P],
                                 rhs=ao23[:, ch * CHW:(ch + 1) * CHW], start=False, stop=True)
                so = outp.tile([P, CHW], bf16, tag="so")
                if eng == "scalar" or (eng is None and dot % 2 == 0):
                    nc.scalar.copy(out=so[:], in_=po[:])
                else:
                    nc.vector.tensor_copy(so[:], po[:])
                nc.sync.dma_start(out=out_d[b, dot, :, ch, :], in_=so[:])

            # Final-chunk split o-proj: the wo0^T ao01 halves run as single
            # matmuls (bf16-staged to SBUF) as fillers inside the LAST strip,
            # filling the PE right after pair0's norms; the tail then only
            # runs the wo1^T ao23 halves, with the staged half added during
            # the bf16 evacuation (no extra copy).
            stage_po = {}

            def emit_oproj_mm1(b, ch, dot, tag):
                ao01 = tiles[b][4]
                po = ps_sm.tile([P, CHW], f32, name="po1", tag=tag)
                nc.tensor.matmul(po[:], lhsT=wo_sb[:, 0, dot * P:(dot + 1) * P],
                                 rhs=ao01[:, ch * CHW:(ch + 1) * CHW],
                                 start=True, stop=True)
                stg = outp.tile([P, CHW], bf16, name="stg", tag="stg", bufs=16)
                if dot % 2 == 0:
                    nc.scalar.copy(out=stg[:], in_=po[:])
                else:
                    nc.vector.tensor_copy(stg[:], po[:])
                stage_po[(b, ch, dot)] = stg

            def oproj_chunk_final(b, ch):
                ao23 = tiles[b][5]
                for dot in range(0, ST, 2):
                    so2 = outp.tile([P, 2, CHW], bf16, tag="so2")
                    pos = []
                    for j in range(2):
                        pos.append(ps_sm.tile([P, CHW], f32, name=f"pof{j}",
                                   tag=("pm" if j == 0 else "tp")))
                        nc.tensor.matmul(
                            pos[j][:],
                            lhsT=wo_sb[:, 1, (dot + j) * P:(dot + j + 1) * P],
                            rhs=ao23[:, ch * CHW:(ch + 1) * CHW],
                            start=True, stop=True)
                    s0 = stage_po.pop((b, ch, dot))
                    s1 = stage_po.pop((b, ch, dot + 1))
                    nc.scalar.activation(so2[:, 0, :], pos[0][:],
                                         mybir.ActivationFunctionType.Copy,
                                         bias=s0[:])
                    nc.vector.tensor_add(so2[:, 1, :], pos[1][:], s1[:])
                    a = out_d[b, dot:dot + 2, :, ch, :]
                    nc.sync.dma_start(
                        out=AP(tensor=a.tensor, offset=a.offset,
                               ap=[a.ap[1], a.ap[0], a.ap[2]]),
                        in_=so2[:])

            def oproj_chunk(b, ch):
                # Final chunk, 3 dot-pairs in flight: pm/tp banks plus one
                # dead scores slot (2 banks). All wo0-half matmuls of the
                # in-flight window are emitted BEFORE any norm-gated wo1
                # half, so the in-order PE queue fills the last strip's norm
                # latency (~4us) instead of idling behind 2 open banks.
                ao01, ao23 = tiles[b][4], tiles[b][5]
                npair = ST // 2
                banks = {}

                def get_banks(pp):
                    if pp % 3 == 2:
                        big = ps_big.tile([P, 2, CHW], f32, name="pobig",
                                          tag="big")
                        return (big[:, 0, :], big[:, 1, :])
                    a = ps_sm.tile([P, CHW], f32, name="poa",
                                   tag=("pm" if pp % 3 == 0 else "tp"))
                    b2 = ps_sm.tile([P, CHW], f32, name="pob",
                                    tag=("tp" if pp % 3 == 0 else "pm"))
                    return (a[:], b2[:])

                def emit_wo0(pp):
                    banks[pp] = get_banks(pp)
                    for j in range(2):
                        dd = 2 * pp + j
                        nc.tensor.matmul(
                            banks[pp][j],
                            lhsT=wo_sb[:, 0, dd * P:(dd + 1) * P],
                            rhs=ao01[:, ch * CHW:(ch + 1) * CHW],
                            start=True, stop=False)

                def emit_wo1(pp):
                    pos = banks.pop(pp)
                    for j in range(2):
                        dd = 2 * pp + j
                        nc.tensor.matmul(
                            pos[j],
                            lhsT=wo_sb[:, 1, dd * P:(dd + 1) * P],
                            rhs=ao23[:, ch * CHW:(ch + 1) * CHW],
                            start=False, stop=True)
                    so2 = outp.tile([P, 2, CHW], bf16, tag="so2")
                    nc.scalar.copy(out=so2[:, 0, :], in_=pos[0])
                    nc.scalar.copy(out=so2[:, 1, :], in_=pos[1])
                    a = out_d[b, 2 * pp:2 * pp + 2, :, ch, :]
                    nc.sync.dma_start(
                        out=AP(tensor=a.tensor, offset=a.offset,
                               ap=[a.ap[1], a.ap[0], a.ap[2]]),
                        in_=so2[:])

                for pp in range(npair):
                    emit_wo0(pp)
                    if pp >= 2:
                        emit_wo1(pp - 2)
                emit_wo1(npair - 2)
                emit_wo1(npair - 1)

            def strip(b, pair, ch, filler=()):
                qt01, qt23, ktd, v1, ao01, ao23 = tiles[b]
                qt, ao = (qt01, ao01) if pair == 0 else (qt23, ao23)
                nks = 4 * (ch + 1)
                filler = list(filler)
                fsched = [[] for _ in range(nks)]
                for i, fd in enumerate(filler):
                    fsched[(i * nks) // len(filler)].append(fd)
                ppr = []
                # per-head single-bank u tiles: pair N+1's PV h0 only waits
                # for pair N's norm_head(h0), not the whole norm
                u0 = ps_u.tile([P, CHW], f32, tag="u0")
                u1 = ps_u.tile([P, CHW], f32, tag="u1")
                uh = (u0, u1)
                DLY = 4

                def emit_pv(kst):
                    lo = max(0, kst - 4 * ch) * P
                    for h in range(2):
                        nc.tensor.matmul(uh[h][:, lo:CHW], lhsT=v1[:, kst, :],
                                         rhs=ppr[kst][:, h, lo:CHW],
                                         start=(kst == 0), stop=(kst == nks - 1))

                def emit_scores(kst):
                    o = kst - 4 * ch
                    lo = max(0, o) * P  # first live qs column of this k-tile
                    psc = ps_big.tile([P, 2, CHW], f32, tag="big")
                    for h in range(2):
                        nc.tensor.matmul(
                            psc[:, h, lo:CHW],
                            lhsT=ktd[64 * h:64 * h + 64, kst * P:(kst + 1) * P],
                            rhs=qt[64 * h:64 * h + 64, ch * CHW + lo:(ch + 1) * CHW],
                            start=True, stop=True)
                    return psc, lo, o

                def emit_exp(kst, psc, lo, o):
                    pt = pp.tile([P, 2, CHW], bf16, tag="p")
                    nc.scalar.activation(pt[:, :, lo:CHW], psc[:, :, lo:CHW],
                                         mybir.ActivationFunctionType.Exp, scale=0.125)
                    if o >= 0:
                        nc.gpsimd.affine_select(
                            out=pt[:, :, lo:lo + P], in_=pt[:, :, lo:lo + P],
                            compare_op=mybir.AluOpType.is_ge,
                            fill=0.0, base=0, channel_multiplier=-1,
                            pattern=[[0, 2], [1, P]],
                        )
                    ppr.append(pt)

                # Process k-tiles in pairs: both PV pairs, then both scores
                # pairs, then both exps. This halves the number of
                # stationary-swap transitions between the 64-deep scores
                # matmuls and the 128-deep PV matmuls (~200ns each).
                for k0 in range(0, nks, 2):
                    for kk in (k0, k0 + 1):
                        if kk >= DLY:
                            emit_pv(kk - DLY)
                    sa = emit_scores(k0)
                    sb = emit_scores(k0 + 1)
                    emit_exp(k0, *sa)
                    emit_exp(k0 + 1, *sb)
                    for fd in fsched[k0] + fsched[k0 + 1]:
                        fd()

                def norm_head(u, basep):
                    # Denominators arrive replicated on partitions 64:128 (64
                    # ones-columns in v1), so a DVE 32x32 block transpose can
                    # spread them across partitions directly; the reciprocal
                    # then runs 16/lane instead of 512 on one lane.
                    tq = normu.tile([64, CHW], f32, tag="tq")
                    nc.vector.transpose(tq[:], u[64:128, :])
                    rt = norm.tile([64, 16], f32, tag="rt")
                    tqa = tq[:]
                    nc.vector.reciprocal(
                        rt[:], AP(tensor=tqa.tensor, offset=tqa.offset,
                                  ap=[tqa.ap[0], [32, 16]]))
                    rb = normu.tile([64, CHW], f32, tag="rb")
                    rta, rba = rt[:], rb[:]
                    nc.vector.transpose(
                        AP(tensor=rba.tensor, offset=rba.offset,
                           ap=[rba.ap[0], [32, 16], [1, 32]]),
                        AP(tensor=rta.tensor, offset=rta.offset,
                           ap=[rta.ap[0], [1, 16], [0, 32]]))
                    nc.vector.tensor_mul(
                        ao[basep:basep + 64, ch * CHW:(ch + 1) * CHW],
                        u[0:64, :], rb[:])

                # Tail: finish h0's PVs first so h0's norm (DVE) overlaps
                # h1's remaining PV matmuls on the PE.
                def emit_pv_h(kst, h):
                    lo = max(0, kst - 4 * ch) * P
                    nc.tensor.matmul(uh[h][:, lo:CHW], lhsT=v1[:, kst, :],
                                     rhs=ppr[kst][:, h, lo:CHW],
                                     start=(kst == 0), stop=(kst == nks - 1))

                for kst in range(max(0, nks - DLY), nks):
                    emit_pv_h(kst, 0)
                norm_head(u0[:], 0)
                for kst in range(max(0, nks - DLY), nks):
                    emit_pv_h(kst, 1)
                norm_head(u1[:], 64)

            # ---- emission schedule ----
            # Ascending chunks pipeline phase A into the strips: strip (b, ch)
            # only needs A(b) s-tiles 0..4ch+3, so later A s-tiles stream in as
            # PE filler one chunk ahead of their first use.
            astiles = [(b, st) for b in range(B) for st in range(ST)]
            emit_astile_mm(0, 0, split_dma=True)
            emit_late_dmas()
            emit_astile_mm(0, 1)
            emit_astile_tp(0, 0)
            emit_astile_mm(0, 2)
            emit_astile_cp(0, 0)
            emit_astile_tp(0, 1)
            emit_astile_mm(0, 3)
            emit_astile_cp(0, 1)
            emit_astile_tp(0, 2)
            emit_astile_cp(0, 2)
            emit_astile_tp(0, 3)
            emit_astile_cp(0, 3)
            ai = 4
            prev = None
            for b in range(B):
                for ch in (0, 1, 2, 3):
                    # A s-tiles needed one chunk ahead (next strip in sequence)
                    if b == 0 and ch < 3:
                        need = 4 + 4 * (ch + 1) + 4
                    elif b == 0:
                        need = ST + 6
                    elif ch < 3:
                        need = ST + min(ST, 4 * (ch + 1) + 4)
                    else:
                        need = 2 * ST
                    take = astiles[ai:need]
                    ai = max(ai, need)
                    for pair in (0, 1):
                        wo_here = (b == 0 and ch == 0 and pair == 1)
                        dots = []
                        if prev is not None:
                            pb, pch = prev
                            alld = [(pb, pch, d) for d in range(ST)]
                            dots = alld[:8] if pair == 0 else alld[8:]
                            if b == 1 and ch == 3 and pair == 1:
                                # hold back 4 dots: emitted after this strip,
                                # they fill the PE during the final norms
                                deferred_dots = dots[4:]
                                dots = dots[:4]
                        ta = (take[:(len(take) + 1) // 2] if pair == 0
                              else take[(len(take) + 1) // 2:])
                        # Interleave astile mm / oproj dots / tp / cp so the
                        # PE always has dot matmuls queued between an astile's
                        # QKV matmuls and its transposes (which wait on rope).
                        # Dot PSUM tags are positional: a dot emitted right
                        # after mm uses the tp bank (its last user was evac'd
                        # a whole astile ago); dots after cp alternate.
                        filler = []
                        di = 0

                        last = (b == 1 and ch == 3)

                        def dot_f(tag):
                            nonlocal di
                            if di < len(dots):
                                filler.append(
                                    lambda t=dots[di], g=tag,
                                    e=("scalar" if last else None):
                                    emit_oproj_dot(*t, tag=g, eng=e))
                                di += 1
                        if not dots:
                            # No dot matmuls to pad with: emit all the mm
                            # parts first (each covers the previous one's
                            # rope latency), then the tp/cp chains.
                            for (ab, ast) in ta:
                                filler.append(
                                    lambda ab=ab, ast=ast: emit_astile_mm(ab, ast))
                            if wo_here:
                                # behind this strip's xt tiles in the queue;
                                # first read one strip later
                                filler.append(lambda: nc.sync.dma_start(
                                    out=wo_sb[:], in_=wo_d[:]))
                            for (ab, ast) in ta:
                                filler.append(
                                    lambda ab=ab, ast=ast: emit_astile_tp(ab, ast))
                                filler.append(
                                    lambda ab=ab, ast=ast: emit_astile_cp(ab, ast))
                            ta = []
                        for (ab, ast) in ta:
                            filler.append(lambda ab=ab, ast=ast: emit_astile_mm(ab, ast))
                            dot_f("tp")
                            dot_f("pm")
                            filler.append(lambda ab=ab, ast=ast: emit_astile_tp(ab, ast))
                            filler.append(lambda ab=ab, ast=ast: emit_astile_cp(ab, ast))
                            # pm first: the tp bank's evac copies (cp, just
                            # emitted) are still draining through the queues
                            dot_f("pm")
                            dot_f("tp")
                        for j, t in enumerate(dots[di:]):
                            filler.append(lambda t=t, g=("tp" if j % 2 == 0 else "pm"):
                                          emit_oproj_dot(*t, tag=g))
                        strip(b, pair, ch, filler=filler)
                    prev = (b, ch)
            for j, t in enumerate(deferred_dots):
                emit_oproj_dot(*t, tag=("pm" if j % 2 == 0 else "tp"),
                               eng="scalar")
            oproj_chunk(*prev)

    nc.compile()
    return nc


def get_nc():
    global _nc_cache
    if _nc_cache is None:
        _nc_cache = build_nc()
    return _nc_cache


def prep_inputs(x, freqs_cos, freqs_sin, wq, wk, wv, wo):
    """Host-side layout prep. Returns list of per-core input dicts."""
    bf = ml_dtypes.bfloat16
    x = np.asarray(x, dtype=np.float32)
    # xh[b, st, p, ct, sl] = x[b, st*128+sl, ct*128+p]
    xh = np.ascontiguousarray(
        x.reshape(B, ST, P, CT, P).transpose(0, 1, 4, 3, 2).astype(bf))
    # fc[p, st, j] = freqs_cos[st*128+p, j]
    fc = np.ascontiguousarray(
        np.asarray(freqs_cos, np.float32).reshape(ST, P, 32).transpose(1, 0, 2))
    fs = np.ascontiguousarray(
        np.asarray(freqs_sin, np.float32).reshape(ST, P, 32).transpose(1, 0, 2))
    perm = np.concatenate([np.arange(0, HD, 2), np.arange(1, HD, 2)])
    in_maps = []
    for c in range(NCORE):
        q_rows = np.asarray(wq, np.float32)[c * HPC * HD:(c + 1) * HPC * HD]
        q_rows = q_rows.reshape(HPC, HD, DIM)[:, perm, :].reshape(HPC * HD, DIM)
        k_rows = np.asarray(wk, np.float32)[c * HD:(c + 1) * HD][perm]
        v_rows = np.asarray(wv, np.float32)[c * HD:(c + 1) * HD]
        wcat = np.concatenate([q_rows, k_rows, v_rows], axis=0)  # [384, DIM]
        w_h = np.ascontiguousarray(wcat.T.reshape(CT, P, QKV).transpose(1, 0, 2).astype(bf))
        wo_cols = np.asarray(wo, np.float32)[:, c * HPC * HD:(c + 1) * HPC * HD]  # [DIM, 256]
        wo_h = np.ascontiguousarray(wo_cols.T.reshape(2, P, DIM).transpose(1, 0, 2).astype(bf))
        in_maps.append({"xt": xh, "wqkv": w_h, "wo": wo_h, "fcos": fc, "fsin": fs})
    return in_maps


def combine_outputs(results):
    """Sum per-core partial out^T and return [B, S, DIM] float32."""
    acc = np.zeros((B, ST, P, NCH, CHW), np.float32)
    for r in results:
        acc += r["out"].astype(np.float32)
    # out[b, ch*512+sl, dot*128+p] = acc[b, dot, p, ch, sl]
    return np.ascontiguousarray(
        acc.transpose(0, 3, 4, 1, 2).reshape(B, S, DIM).astype(np.float32))


def kernel(x, freqs_cos, freqs_sin, wq, wk, wv, wo):
    from concourse.bass_utils import run_bass_kernel_spmd

    nc = get_nc()
    in_maps = prep_inputs(x, freqs_cos, freqs_sin, wq, wk, wv, wo)
    res = run_bass_kernel_spmd(nc, in_maps, core_ids=list(range(NCORE)))
    return combine_outputs(res.results)

